# revision 1
# baseline (speedup 1.0000x reference)
"""Canny filter (blur -> sobel -> orientation-quantized NMS) on 8 Trainium2 cores.

Self-contained: batch 16 is sharded 2 images/core (pure data parallel);
each core runs an identical Bass/Tile program on its [2,3,512,512] slice.
"""
import sys
import numpy as np

sys.path.insert(0, "/opt/trn_rl_repo")

import concourse.bacc as bacc
import concourse.tile as tile
from concourse import mybir
from concourse.bass_utils import run_bass_kernel_spmd
from contextlib import ExitStack

F32 = mybir.dt.float32
U8 = mybir.dt.uint8

B, C, H, W = 16, 3, 512, 512
N_CORES = 8
B_PER = B // N_CORES          # 2 images per core
P = 128                       # partitions per tile
WP = W + 2                    # padded width
# row-tile input origins per image; tile t covers input rows [R, R+128),
# valid output rows are [R+3, R+125)
R_INS = [-3, 119, 241, 363, 387]

_ALU = mybir.AluOpType
_ACTF = mybir.ActivationFunctionType


def _banded(diag_vals, fold_top=False, fold_bot=False, zero_top=False, zero_bot=False):
    """lhsT[k, m] matrix for out[m] = sum_dk w[dk] * in[m+dk], dk in {-1,0,1}.

    fold_top: column m=3 treats in[2] as in[3] (edge replication at image top).
    fold_bot: column m=124 treats in[125] as in[124].
    zero_top/zero_bot: drop the out-of-image tap entirely (NMS zero padding).
    """
    wm1, w0, wp1 = diag_vals
    A = np.zeros((P, P), np.float64)
    for m in range(P):
        for dk, wv in ((-1, wm1), (0, w0), (1, wp1)):
            k = m + dk
            if 0 <= k < P and wv != 0.0:
                A[k, m] += wv
    if fold_top:
        A[2, 3] = 0.0
        A[3, 3] = w0 + wm1
    if zero_top:
        A[2, 3] = 0.0
    if fold_bot:
        A[125, 124] = 0.0
        A[124, 124] = w0 + wp1
    if zero_bot:
        A[125, 124] = 0.0
    return A.astype(np.float32)


def _build_weights():
    v = np.array([np.exp(-0.5), 1.0, np.exp(-0.5)], np.float64)
    sv = v.sum()
    g1 = v / sv                      # vertical gaussian taps
    h = v / (3.0 * sv)               # horizontal gaussian taps (folds the /C)
    ws = {}
    ws["Vg"] = _banded((g1[0], g1[1], g1[2]))
    ws["Ih0"] = (np.eye(P) * h[0]).astype(np.float32)
    ws["Ih1"] = (np.eye(P) * h[1]).astype(np.float32)
    # sobel (scaled by 2 -> integer taps): Gx = [1,2,1]_v (x) [-1,0,1]_h,
    # Gy = [-1,0,1]_v (x) [1,2,1]_h   (vertical +1 tap is the row below)
    for suf, kw in (("", {}), ("_t", {"fold_top": True}), ("_b", {"fold_bot": True})):
        ws["Vs" + suf] = _banded((1.0, 2.0, 1.0), **kw)
        ws["Vsn" + suf] = -ws["Vs" + suf]
        ws["Vd" + suf] = _banded((-1.0, 0.0, 1.0), **kw)
        ws["Vd2" + suf] = 2.0 * ws["Vd" + suf]
    ws["ShN"] = _banded((1.0, 0.0, 0.0))
    ws["ShN_t"] = _banded((1.0, 0.0, 0.0), zero_top=True)
    ws["ShS"] = _banded((0.0, 0.0, 1.0))
    ws["ShS_b"] = _banded((0.0, 0.0, 1.0), zero_bot=True)
    return ws

# angle-band thresholds: tan((2j-1)*pi/16)^2, j=1..4
_TJ2 = [float(np.tan((2 * j - 1) * np.pi / 16.0) ** 2) for j in (1, 2, 3, 4)]

_NC_CACHE = None


def _build_program():
    nc = bacc.Bacc("TRN2", target_bir_lowering=False, debug=False, num_devices=N_CORES)
    img = nc.declare_dram_parameter("img", [B_PER, C, H, W], F32, isOutput=False)
    out = nc.declare_dram_parameter("out", [B_PER, 1, H, W], F32, isOutput=True)

    wnp = _build_weights()
    wdram = {k: nc.inline_tensor(v, name="w_" + k) for k, v in wnp.items()}

    with tile.TileContext(nc) as tc, ExitStack() as ctx:
        cpool = ctx.enter_context(tc.tile_pool(name="consts", bufs=1))
        pool = ctx.enter_context(tc.tile_pool(name="work", bufs=2))
        mpool = ctx.enter_context(tc.tile_pool(name="masks", bufs=2))
        pp = ctx.enter_context(tc.tile_pool(name="ps", bufs=1, space="PSUM"))

        wsb = {}
        for k in wnp:
            wt = cpool.tile([P, P], F32, tag="w_" + k)
            nc.sync.dma_start(wt[:], wdram[k][:])
            wsb[k] = wt
        zero = cpool.tile([P, W], F32, tag="zero")
        nc.vector.memset(zero[:], 0.0)

        for i in range(B_PER):
            for t_i, R in enumerate(R_INS):
                top = t_i == 0
                bot = t_i == len(R_INS) - 1
                suf = "_t" if top else ("_b" if bot else "")

                # ---- load channels (with replicated clamp rows at image edges)
                cs = []
                for k in range(C):
                    ct = pool.tile([P, W], F32, tag=f"c{k}")
                    if top:
                        nc.sync.dma_start(ct[0:3, :], img[i, k, 0:1, :].broadcast_to((3, W)))
                        nc.sync.dma_start(ct[3:128, :], img[i, k, 0:125, :])
                    elif bot:
                        nc.sync.dma_start(ct[0:125, :], img[i, k, R:R + 125, :])
                        nc.sync.dma_start(ct[125:128, :], img[i, k, 511:512, :].broadcast_to((3, W)))
                    else:
                        nc.sync.dma_start(ct[:], img[i, k, R:R + 128, :])
                    cs.append(ct)

                # ---- vertical gauss + channel sum (PE)
                ps_u = pp.tile([P, W], F32, tag="u")
                for k in range(C):
                    nc.tensor.matmul(ps_u[:], wsb["Vg"][:], cs[k][:],
                                     start=(k == 0), stop=(k == C - 1))
                u = pool.tile([P, WP], F32, tag="u_sb")
                nc.scalar.copy(u[:, 1:1 + W], ps_u[:])
                nc.vector.tensor_copy(u[:, 0:1], u[:, 1:2])
                nc.vector.tensor_copy(u[:, WP - 1:WP], u[:, W:W + 1])

                # ---- horizontal gauss (PE, identity-scaled shifted rhs)
                ps_t = pp.tile([P, W], F32, tag="t")
                nc.tensor.matmul(ps_t[:], wsb["Ih0"][:], u[:, 0:W], start=True, stop=False)
                nc.tensor.matmul(ps_t[:], wsb["Ih1"][:], u[:, 1:1 + W], start=False, stop=False)
                nc.tensor.matmul(ps_t[:], wsb["Ih0"][:], u[:, 2:2 + W], start=False, stop=True)
                tt = pool.tile([P, WP], F32, tag="t_sb")
                nc.scalar.copy(tt[:, 1:1 + W], ps_t[:])
                nc.vector.tensor_copy(tt[:, 0:1], tt[:, 1:2])
                nc.vector.tensor_copy(tt[:, WP - 1:WP], tt[:, W:W + 1])

                # ---- sobel (PE)
                ps_gx = pp.tile([P, W], F32, tag="gx")
                nc.tensor.matmul(ps_gx[:], wsb["Vsn" + suf][:], tt[:, 0:W], start=True, stop=False)
                nc.tensor.matmul(ps_gx[:], wsb["Vs" + suf][:], tt[:, 2:2 + W], start=False, stop=True)
                ps_gy = pp.tile([P, W], F32, tag="gy")
                nc.tensor.matmul(ps_gy[:], wsb["Vd" + suf][:], tt[:, 0:W], start=True, stop=False)
                nc.tensor.matmul(ps_gy[:], wsb["Vd2" + suf][:], tt[:, 1:1 + W], start=False, stop=False)
                nc.tensor.matmul(ps_gy[:], wsb["Vd" + suf][:], tt[:, 2:2 + W], start=False, stop=True)

                # ---- squares, sign product
                sqx = pool.tile([P, W], F32, tag="sqx")
                nc.scalar.activation(sqx[:], ps_gx[:], _ACTF.Square)
                gy = pool.tile([P, W], F32, tag="gy_sb")
                nc.scalar.copy(gy[:], ps_gy[:])
                prod = pool.tile([P, W], F32, tag="prod")
                nc.vector.scalar_tensor_tensor(prod[:], ps_gx[:], 1.0, gy[:],
                                               _ALU.mult, _ALU.mult)
                sqy = pool.tile([P, W], F32, tag="sqy")
                nc.vector.tensor_tensor(sqy[:], gy[:], gy[:], _ALU.mult)

                msq = pool.tile([P, WP], F32, tag="msq")
                nc.vector.tensor_tensor(msq[:, 1:1 + W], sqx[:], sqy[:], _ALU.add)
                nc.vector.memset(msq[:, 0:1], 0.0)
                nc.vector.memset(msq[:, WP - 1:WP], 0.0)

                # ---- N/S shifted copies of msq (PE shift matmuls)
                ps_n = pp.tile([P, W], F32, tag="nsh")
                nc.tensor.matmul(ps_n[:], wsb["ShN_t" if top else "ShN"][:],
                                 msq[:, 1:1 + W], start=True, stop=True)
                ps_s = pp.tile([P, W], F32, tag="ssh")
                nc.tensor.matmul(ps_s[:], wsb["ShS_b" if bot else "ShS"][:],
                                 msq[:, 1:1 + W], start=True, stop=True)
                nsb = pool.tile([P, WP], F32, tag="nsb")
                nc.scalar.copy(nsb[:, 1:1 + W], ps_n[:])
                nc.vector.memset(nsb[:, 0:1], 0.0)
                nc.vector.memset(nsb[:, WP - 1:WP], 0.0)
                ssb = pool.tile([P, WP], F32, tag="ssb")
                nc.scalar.copy(ssb[:, 1:1 + W], ps_s[:])
                nc.vector.memset(ssb[:, 0:1], 0.0)
                nc.vector.memset(ssb[:, WP - 1:WP], 0.0)

                # ---- orientation band masks c1..c4, sign, diagonal choice q
                cms = []
                for j, tj2 in enumerate(_TJ2):
                    cm = mpool.tile([P, W], U8, tag=f"c{j}m")
                    nc.vector.scalar_tensor_tensor(cm[:], sqx[:], tj2, sqy[:],
                                                   _ALU.mult, _ALU.is_lt)
                    cms.append(cm)
                s01 = mpool.tile([P, W], U8, tag="s01")
                nc.vector.tensor_scalar(s01[:], prod[:], 0.0, None, _ALU.is_gt)
                q = mpool.tile([P, W], U8, tag="q")
                nc.vector.tensor_tensor(q[:], s01[:], cms[2][:], _ALU.not_equal)

                # ---- neighbor maxes along each axis
                a1 = pool.tile([P, W], F32, tag="a1")   # NE / SW
                nc.vector.tensor_tensor(a1[:], nsb[:, 2:2 + W], ssb[:, 0:W], _ALU.max)
                a3 = pool.tile([P, W], F32, tag="a3")   # NW / SE
                nc.vector.tensor_tensor(a3[:], nsb[:, 0:W], ssb[:, 2:2 + W], _ALU.max)
                adiag = pool.tile([P, W], F32, tag="adiag")
                nc.scalar.copy(adiag[:], a3[:])
                nc.vector.copy_predicated(adiag[:], q[:], a1[:])
                a2 = pool.tile([P, W], F32, tag="a2")   # N / S
                nc.vector.tensor_tensor(a2[:], nsb[:, 1:1 + W], ssb[:, 1:1 + W], _ALU.max)
                m = pool.tile([P, W], F32, tag="m")     # E / W
                nc.vector.tensor_tensor(m[:], msq[:, 0:W], msq[:, 2:2 + W], _ALU.max)
                a0c = pool.tile([P, W], F32, tag="a0c")
                nc.scalar.copy(a0c[:], m[:])
                nc.vector.copy_predicated(m[:], cms[0][:], adiag[:])
                nc.vector.copy_predicated(m[:], cms[1][:], a2[:])
                nc.vector.copy_predicated(m[:], cms[2][:], adiag[:])
                nc.vector.copy_predicated(m[:], cms[3][:], a0c[:])

                # ---- remove mask + apply + sqrt
                rm = mpool.tile([P, W], U8, tag="rm")
                nc.vector.tensor_tensor(rm[:], m[:], msq[:, 1:1 + W], _ALU.is_ge)
                nc.vector.copy_predicated(msq[:, 1:1 + W], rm[:], zero[:])
                osb = pool.tile([P, W], F32, tag="osb")
                nc.scalar.activation(osb[:], msq[:, 1:1 + W], _ACTF.Sqrt, scale=0.25)

                r0, r1 = R + 3, R + 125
                nc.sync.dma_start(out[i, 0, r0:r1, :], osb[3:125, :])

    nc.compile()
    return nc


def _get_program():
    global _NC_CACHE
    if _NC_CACHE is None:
        _NC_CACHE = _build_program()
    return _NC_CACHE


def kernel(img, w_gauss=None, w_sobel_x=None, w_sobel_y=None, w_dir=None):
    img = np.ascontiguousarray(np.asarray(img, dtype=np.float32))
    assert img.shape == (B, C, H, W)
    nc = _get_program()
    in_maps = [{"img": img[c * B_PER:(c + 1) * B_PER]} for c in range(N_CORES)]
    res = run_bass_kernel_spmd(nc, in_maps, list(range(N_CORES)))
    return np.concatenate([res.results[c]["out"] for c in range(N_CORES)], axis=0)
